# revision 28
# baseline (speedup 1.0000x reference)
"""Trainium2 Bass kernel for nn_AttentionBlock (linear attention block).

Data-parallel over batch: core b computes batch b end-to-end (no collectives).

Math (per batch, heads h=8, dh=64, T=4096, F=256):
  qkv = w_qkv^T @ x                         (channel layout interleaved d*24+h*3+n)
  q,k = elu(.)+1 ; v raw
  cntxt[h] = k_h @ v_h^T  (contract T)      -> [64,64]
  norm[h,d] = sum_t q_h[d,t]*k_h[d,t]
  val[h]  = cntxt[h]^T @ q_h / (8*norm)     (per-row e scaling)
  out = w_out^T @ val_flat
Kernel folds cntxt, the 1/(8*norm) scaling, and w_out into a small
per-head matrix W2[hd, o] = sum_e cntxt[d,e] * w_out[he,o] / (8*norm[he]),
so the big T-dim epilogue is a single matmul: out = W2^T @ q.

v2 structure (single software-pipelined phase 1, then a short phase 2):

Phase 1, 32 t-tile slots of 128 (each ~1.7us of PE):
  PE   : 4 PE-transposes of q~(tt-2) -> psT; c-outer qkv (c0: v,q,k;
         c1: v,q,k — 2 LDWEIGHTS/tile since v/q/k share the x
         stationary); 4 ctx matmuls for tile tt-3.
  ACT  : Exp(psQ)->e_q, Exp(psK)->e_k (split halves shorten the psum
         WAR chain), v eviction.
  DVE  : psT evict (bf16 2x mode) -> q_sb; ELU1SEL selects for q and k;
         pair-add tree for the norm (2 levels -> 8 groups).
  Pool : q~*k~ product for norm.
  PSUM : psQ(2)+psK(2)+psV(2)+psT(1)+ctx(1) = 8 banks exactly.
Phase 2: 8 norm matmuls; PSUM-direct reciprocal; scaled block-diag
  cntxtT; W2 matmuls.
Phase 3: out = W2^T @ q with w2-stationary reuse, PSUM eviction
  alternating DVE/ACT, contiguous chunked DMA out.

DMA: all transfers are contiguous 128-partition slabs (host repacks x
t-chunk-major, weights per c-chunk [v|q|k], out (oc, t-chunk)-major).
"""

import sys, types

if "/opt/trn_rl_repo" not in sys.path:
    sys.path.insert(0, "/opt/trn_rl_repo")

import numpy as np
import ml_dtypes

# ---------------------------------------------------------------------------
# axon NTFF profile hook stub (lets run_bass_kernel_spmd(trace=True) work; the
# plain untraced path used for grading does not need it, but installing is
# harmless and lets any caller profile).
# ---------------------------------------------------------------------------
def _install_axon_hook_stub():
    try:
        import antenv
        if "antenv.axon_hooks" in sys.modules:
            return
        hooks = types.ModuleType("antenv.axon_hooks")
        hooks._hook = None
        def set_axon_ntff_profile_hook(h):
            hooks._hook = h
        def get_axon_ntff_profile_hook():
            return hooks._hook
        hooks.set_axon_ntff_profile_hook = set_axon_ntff_profile_hook
        hooks.get_axon_ntff_profile_hook = get_axon_ntff_profile_hook
        sys.modules["antenv.axon_hooks"] = hooks
        antenv.axon_hooks = hooks
        try:
            from trn_agent_boot.trn_boot import _ntff_profile_via_ctypes
            hooks._hook = _ntff_profile_via_ctypes("/opt/axon/libaxon_pjrt.so")
        except Exception:
            pass
    except Exception:
        pass

_install_axon_hook_stub()

import concourse.mybir as mybir
import concourse.tile as tile
from concourse import bacc, dve_ops
from concourse.bass_utils import run_bass_kernel_spmd
from concourse.dve_spec import Spec, Src0, Src1, Zero, One, select, lower
from concourse.dve_uop import DveOpSpec
from concourse.masks import make_identity

B, F, T = 8, 256, 4096
NH, DH = 8, 64
HID = NH * DH            # 512
NT = T // 128            # 32 t-tiles
NPAIR = 4                # head pairs (2 heads = 128 channels)
BF16 = mybir.dt.bfloat16
F32 = mybir.dt.float32
AF = mybir.ActivationFunctionType

# ---------------------------------------------------------------------------
# custom DVE op: out = x > 0 ? x+1 : e   (e = exp(x) supplied by ScalarE)
# ---------------------------------------------------------------------------
def _register_elu_select():
    for op in dve_ops.OPS:
        if op.name == "ELU1SEL":
            return op
    spec = Spec(
        body=select(Src0 > Zero, Src0 + One, Src1),
        reference=lambda in0, in1, s0, s1, imm2: np.where(
            in0 > 0, in0.astype(np.float32) + 1.0, in1
        ).astype(np.float32),
    )
    shas = {}
    for ver in ("v3", "v4"):
        uops = lower(spec, ver=ver)
        shas[ver] = DveOpSpec(name="ELU1SEL", opcode=0, uops=uops, rd1_en=True).sha(ver)
    op = dve_ops.DveOp("ELU1SEL", spec, subdim=False, uops_sha=shas)
    dve_ops.OPS.append(op)
    dve_ops.CUSTOM_DVE_SPECS[op.name] = spec
    dve_ops._SUB_OPCODE_FOR_NAME[op.name] = max(dve_ops._SUB_OPCODE_FOR_NAME.values()) + 1
    return op

ELU1SEL = _register_elu_select()


def _build_kernel():
    nc = bacc.Bacc("TRN2", target_bir_lowering=False, debug=False, num_devices=8)

    # contiguous-slab layouts (host repacks; see _prep_weights / kernel)
    x_d = nc.dram_tensor("x", [8, 128, 1024], BF16, kind="ExternalInput")
    wvqk_d = nc.dram_tensor("wvqk", [2, 128, 3 * HID], BF16, kind="ExternalInput")
    wo_d = nc.dram_tensor("wo", [128, 4 * F], BF16, kind="ExternalInput")
    out_d = nc.dram_tensor("out", [2, 8, 128, 512], BF16, kind="ExternalOutput")

    with tile.TileContext(nc) as tc:
        with (
            tc.tile_pool(name="const", bufs=1) as constp,
            tc.tile_pool(name="wts", bufs=1) as wts,
            tc.tile_pool(name="xin", bufs=1) as xin,
            tc.tile_pool(name="qkbuf", bufs=1) as qkbuf,
            tc.tile_pool(name="qbuf", bufs=1) as qbuf,
            tc.tile_pool(name="work", bufs=5) as work,
            tc.tile_pool(name="ostage", bufs=6) as ostage,
        ):
            ones_sb = constp.tile([128, 1], BF16)
            nc.vector.memset(ones_sb[:], 1.0)
            one_f32 = constp.tile([1, 1], F32)
            nc.vector.memset(one_f32[:], 1.0)
            zeros_sb = constp.tile([128, 128], BF16)
            nc.vector.memset(zeros_sb[:], 0.0)
            ident_sb = constp.tile([128, 128], BF16)
            make_identity(nc, ident_sb[:])

            wvqk_sb = wts.tile([128, 2, 3 * HID], BF16)   # per c: [v|q|k]
            wo_sb = wts.tile([128, 4 * F], BF16)
            x_sb = xin.tile([128, 8, 1024], BF16)         # per chunk: [c0 512t|c1 512t]

            # contiguous DMAs, hot-first, split across the two hardware DGE
            # queues (sync + scalar) so the first transfers run in parallel
            # (gpsimd DMA is software-DGE: slow transfers, avoid).  wv|wq of
            # c0 land first (first tile consumes v,q,k in order), then wk.
            nc.sync.dma_start(wvqk_sb[:, 0, 0:2 * HID], wvqk_d.ap()[0][:, 0:2 * HID])
            nc.scalar.dma_start(x_sb[:, 0, :], x_d.ap()[0])
            nc.sync.dma_start(wvqk_sb[:, 0, 2 * HID:3 * HID],
                              wvqk_d.ap()[0][:, 2 * HID:3 * HID])
            nc.scalar.dma_start(wvqk_sb[:, 1, :], wvqk_d.ap()[1])
            nc.sync.dma_start(x_sb[:, 1, :], x_d.ap()[1])
            nc.scalar.dma_start(x_sb[:, 2, :], x_d.ap()[2])
            nc.sync.dma_start(wo_sb[:], wo_d.ap())
            for tch in range(3, 8):
                eng = nc.sync if tch % 2 == 1 else nc.scalar
                eng.dma_start(x_sb[:, tch, :], x_d.ap()[tch])

            def wv(c):
                return wvqk_sb[:, c, 0:HID]
            def wq(c):
                return wvqk_sb[:, c, HID:2 * HID]
            def wk(c):
                return wvqk_sb[:, c, 2 * HID:3 * HID]
            def xsl(tt, c):
                lo = c * 512 + (tt % 4) * 128
                return x_sb[:, tt // 4, lo:lo + 128]

            # persistent activations
            qkT = qkbuf.tile([128, NT, 2 * HID], BF16)    # [:, tt, 0:512]=q~, [512:]=k~
            pT_sb = qkbuf.tile([128, 12, HID], BF16)      # pair sums of q~*k~ (tiles 0-23)
            mtail_sb = qbuf.tile([128, 8, HID], BF16)     # raw products, tiles 24-31
            q_sb = qbuf.tile([128, 4, T], BF16)           # q~[hd, t], hd = c*128+p

            with tc.tile_pool(name="psB", bufs=1, space="PSUM") as psB:
                ctx_ps = psB.tile([128, NPAIR * 128], F32)   # cntxtT pair blocks

                # ---------------- phase 1 ----------------
                with (
                    tc.tile_pool(name="psQ", bufs=2, space="PSUM") as psQ,
                    tc.tile_pool(name="psK", bufs=2, space="PSUM") as psK,
                    tc.tile_pool(name="psV", bufs=2, space="PSUM") as psV,
                    tc.tile_pool(name="psT", bufs=1, space="PSUM") as psT,
                ):
                    vt_of = {}
                    m_of = {}

                    def emit_transpose(tt):
                        # PE transposes of q~(tt) into one psum bank; DVE
                        # evicts (bf16 psum -> 2x mode) into q_sb.  The last
                        # few go to ACT instead: it idles in the tail while
                        # the DVE tail chain feeds the norm.
                        pt = psT.tile([128, 4, 128], BF16, tag="tp")
                        for c in range(4):
                            nc.tensor.transpose(
                                pt[:, c, :],
                                qkT[:, tt, c * 128:(c + 1) * 128],
                                ident_sb[:])
                        dst = q_sb[:, :, tt * 128:(tt + 1) * 128]
                        if tt >= 28:
                            nc.scalar.activation(dst, pt[:], AF.Copy)
                        else:
                            nc.vector.tensor_copy(dst, pt[:])

                    def emit_qkv(tt):
                        psq = psQ.tile([128, HID], F32, tag="pq")
                        psk = psK.tile([128, HID], F32, tag="pk")
                        pv = psV.tile([128, HID], F32, tag="pv")
                        # c-outer: v/q/k share the x stationary (2 LDW/tile);
                        # v first so its (short) evict chain gates earliest.
                        for c in range(2):
                            xs = xsl(tt, c)
                            nc.tensor.matmul(pv[:], xs, wv(c),
                                             start=(c == 0), stop=(c == 1))
                            nc.tensor.matmul(psq[:], xs, wq(c),
                                             start=(c == 0), stop=(c == 1))
                            nc.tensor.matmul(psk[:], xs, wk(c),
                                             start=(c == 0), stop=(c == 1))

                        # split elu: per half, ACT Exp then DVE select.  The
                        # split matters beyond WAR slack: the PE transpose of
                        # q~(tt) two slots later waits only on sel_q, which
                        # retires ~700ns before sel_k, keeping the PE<->DVE
                        # round-trip off the critical path.
                        e_q = work.tile([128, HID], BF16, tag="eq")
                        e_k = work.tile([128, HID], BF16, tag="ek")
                        nc.scalar.activation(e_q[:], psq[:], AF.Exp)
                        nc.scalar.activation(e_k[:], psk[:], AF.Exp)
                        nc.vector._custom_dve(
                            ELU1SEL, out=qkT[:, tt, 0:HID], in0=psq[:], in1=e_q[:])
                        nc.vector._custom_dve(
                            ELU1SEL, out=qkT[:, tt, HID:2 * HID], in0=psk[:], in1=e_k[:])

                        # v eviction on ACT (DVE is the busier engine)
                        vt = work.tile([128, HID], BF16, tag="vt")
                        nc.scalar.activation(vt[:], pv[:], AF.Copy)
                        vt_of[tt] = vt

                        # q~*k~ on the otherwise-idle Pool engine.  Tiles
                        # 24-31 write a persistent buffer and skip the pair
                        # tree entirely: the phase-2 norm matmuls read them
                        # directly, so the tail waits only on Pool's last
                        # mul, not a serialized add chain.
                        if tt >= 24:
                            m_t = mtail_sb[:, tt - 24, :]
                            nc.gpsimd.tensor_mul(
                                m_t, qkT[:, tt, 0:HID], qkT[:, tt, HID:2 * HID])
                        else:
                            m_t = work.tile([128, HID], BF16, tag="mt")
                            nc.gpsimd.tensor_mul(
                                m_t[:], qkT[:, tt, 0:HID], qkT[:, tt, HID:2 * HID])
                            m_of[tt] = m_t

                    def emit_pairadd(tt):
                        # pair-sum tree for norm (tiles 0-23 only),
                        # alternating Pool/DVE (both are near-saturated;
                        # split the load).  Deps are >=2 slots old so
                        # neither in-order engine waits.
                        if tt >= 3 and (tt - 3) % 2 == 0 and tt <= 25:
                            g = (tt - 3) // 2
                            eng = nc.gpsimd if g % 2 == 0 else nc.vector
                            eng.tensor_add(
                                pT_sb[:, g, :], m_of[tt - 3][:], m_of[tt - 2][:])
                            del m_of[tt - 3], m_of[tt - 2]

                    def emit_ctx(tt):
                        kt = qkT[:, tt, HID:2 * HID]
                        if tt == 0:
                            # start=True clears has_written for the WHOLE bank,
                            # so it must happen exactly once for the shared ctx
                            # bank: write zeros across all 4 pair slots, then
                            # only accumulate.
                            nc.tensor.matmul(ctx_ps[:], zeros_sb[:], kt,
                                             start=True, stop=False)
                        vt = vt_of[tt]
                        for pr in range(NPAIR):
                            sl = slice(pr * 128, (pr + 1) * 128)
                            nc.tensor.matmul(
                                ctx_ps[:, sl], vt[:, sl],
                                qkT[:, tt, HID + pr * 128:HID + (pr + 1) * 128],
                                start=False, stop=(tt == NT - 1))
                        del vt_of[tt]

                    for tt in range(NT + 3):
                        if tt >= 2 and tt - 2 < NT:
                            emit_transpose(tt - 2)
                        if tt < NT:
                            emit_qkv(tt)
                            emit_pairadd(tt)
                        if tt >= 3:
                            emit_ctx(tt - 3)
                # ---------------- phase 2 ----------------
                with (
                    tc.tile_pool(name="psN", bufs=1, space="PSUM") as psN,
                    tc.tile_pool(name="psW", bufs=2, space="PSUM") as psW,
                ):
                    # 12 pair tiles + 8 raw tail products, round-robin over
                    # two psum banks: back-to-back matmul accumulation into
                    # the SAME psum address runs at half rate (RMW
                    # recovery), so alternating banks keeps 216ns cadence.
                    na = psN.tile([1, HID], F32)
                    nb = psN.tile([1, HID], F32)
                    red = [pT_sb[:, g, :] for g in range(12)]
                    red += [mtail_sb[:, j, :] for j in range(8)]
                    for g, r in enumerate(red):
                        nc.tensor.matmul(
                            (na if g % 2 == 0 else nb)[:], ones_sb[:], r,
                            start=(g < 2), stop=(g >= len(red) - 2))

                    # norm -> rscale = 1/norm transposed to [128, 4] via
                    # 4 tiny PE transposes + a PSUM-direct reciprocal
                    # (the 1/8 factor is pre-folded into wo on the host)
                    # only one PSUM operand allowed per DVE op: ACT evicts
                    # bank a (overlapping bank b's last matmuls), DVE adds.
                    norm_a = constp.tile([1, HID], F32)
                    nc.scalar.activation(norm_a[:], na[:], AF.Copy)
                    norm8 = constp.tile([1, HID], F32)
                    nc.vector.tensor_add(norm8[:], norm_a[:], nb[:])
                    rsc = constp.tile([128, 4], F32)

                    w2_sb = wts.tile([128, NPAIR, F], BF16)
                    ctx_bd = wts.tile([128, NPAIR, 128], BF16)
                    nt_ps = psN.tile([128, 4], F32)
                    for j in range(4):
                        nc.tensor.transpose(
                            nt_ps[:, j:j + 1],
                            norm8[:, j * 128:(j + 1) * 128],
                            one_f32[:])
                    nc.vector.reciprocal(rsc[:], nt_ps[:])

                    # rsc-scaled cntxtT pair blocks (no masking needed: the
                    # W2 matmuls below only consume the diagonal 64x64
                    # quadrants via tile_position); split DVE/ACT so the
                    # four scales run ~pairwise-parallel
                    for pr in range(NPAIR):
                        if pr < 2:
                            nc.vector.tensor_scalar_mul(
                                ctx_bd[:, pr, :],
                                ctx_ps[:, pr * 128:(pr + 1) * 128],
                                rsc[:, pr:pr + 1],
                            )
                        else:
                            nc.scalar.activation(
                                ctx_bd[:, pr, :],
                                ctx_ps[:, pr * 128:(pr + 1) * 128],
                                AF.Copy, scale=rsc[:, pr:pr + 1],
                            )
                    for pr in range(NPAIR):
                        w2_ps = psW.tile([128, F], F32, tag="w2")
                        for hr in range(2):
                            rows = slice(hr * 64, (hr + 1) * 64)
                            nc.tensor.matmul(
                                w2_ps[rows, :],
                                ctx_bd[rows, pr, hr * 64:(hr + 1) * 64],
                                wo_sb[rows, pr * F:(pr + 1) * F],
                                start=True, stop=True,
                                tile_position=(hr * 64, hr * 64))
                        if pr % 2 == 0:
                            nc.vector.tensor_copy(w2_sb[:, pr, :], w2_ps[:])
                        else:
                            nc.scalar.activation(w2_sb[:, pr, :], w2_ps[:], AF.Copy)

            # ---------------- phase 3 ----------------
            # out = W2^T @ q; w2 chunks stay stationary across 4 t-chunks
            # (16 LDWEIGHTS total), PSUM evictions alternate DVE/ACT.
            with tc.tile_pool(name="psO", bufs=4, space="PSUM") as psO:
                for oc in range(2):
                    for g in range(4):
                        po = psO.tile([128, 2, 512], F32, tag="po")
                        for c in range(4):
                            for ti in range(2):
                                tc_i = g * 2 + ti
                                tsl = slice(tc_i * 512, (tc_i + 1) * 512)
                                nc.tensor.matmul(
                                    po[:, ti, :],
                                    w2_sb[:, c, oc * 128:(oc + 1) * 128],
                                    q_sb[:, c, tsl],
                                    start=(c == 0), stop=(c == 3),
                                )
                        for ti in range(2):
                            tc_i = g * 2 + ti
                            # bf16 staging halves the out-DMA traffic; host
                            # converts back to f32.  Evictions and DMA issue
                            # both split across two engines/queues.
                            ot = ostage.tile([128, 512], BF16, tag="ot")
                            if ti == 0:
                                nc.vector.tensor_copy(ot[:], po[:, ti, :])
                                nc.sync.dma_start(out_d.ap()[oc, tc_i], ot[:])
                            else:
                                nc.scalar.activation(ot[:], po[:, ti, :], AF.Copy)
                                nc.scalar.dma_start(out_d.ap()[oc, tc_i], ot[:])

    nc.compile()
    return nc


_NC = None

def _get_nc():
    global _NC
    if _NC is None:
        _NC = _build_kernel()
    return _NC


def _prep_weights(w_qkv, w_out):
    """Host-side: un-interleave qkv columns to [h,d]-major, cast bf16, pack."""
    d = np.arange(DH)[:, None]          # 64
    h = np.arange(NH)[None, :]          # 8
    # channel index in w_qkv for (h, d, n): d*24 + h*3 + n ; we want [h*64+d]
    def cols(n):
        c = (d * (NH * 3) + h * 3 + n)  # [64, 8]
        return c.T.reshape(-1)          # h-major: [h*64+d]
    bf = ml_dtypes.bfloat16
    wq = np.ascontiguousarray(w_qkv[:, cols(0)])
    wk = np.ascontiguousarray(w_qkv[:, cols(1)])
    wv = np.ascontiguousarray(w_qkv[:, cols(2)])
    # [2, 128, 1536]: per c-chunk [v|q|k]
    wvqk = np.empty((2, 128, 3 * HID), dtype=np.float32)
    for c in range(2):
        rows = slice(c * 128, (c + 1) * 128)
        wvqk[c] = np.hstack([wv[rows], wq[rows], wk[rows]])
    wvqk = wvqk.astype(bf)
    # [128, 4*256]: row p holds w_out[j*128+p, :] for j=0..3; the attention
    # 1/sqrt(DH)=1/8 scale is folded in here (exact in bf16: power of two)
    wo = np.ascontiguousarray(
        (w_out * 0.125).reshape(4, 128, F).transpose(1, 0, 2).reshape(128, 4 * F)
    ).astype(bf)
    return wvqk, wo


def _prep_x(xb):
    """[256, 4096] f32 -> [8, 128, 1024] bf16, per chunk [c0 512t | c1 512t]."""
    bf = ml_dtypes.bfloat16
    return np.ascontiguousarray(
        xb.reshape(2, 128, 8, 512).transpose(2, 1, 0, 3).reshape(8, 128, 1024)
    ).astype(bf)


def _unpack_out(r):
    """[2, 8, 128, 512] bf16 -> [256, 4096] f32."""
    return np.ascontiguousarray(
        r.astype(np.float32).transpose(0, 2, 1, 3).reshape(F, T))


def kernel(x, w_qkv, w_out):
    x = np.asarray(x, dtype=np.float32)
    w_qkv = np.asarray(w_qkv, dtype=np.float32)
    w_out = np.asarray(w_out, dtype=np.float32)
    nc = _get_nc()
    wvqk, wo = _prep_weights(w_qkv, w_out)
    in_maps = []
    for b in range(B):
        in_maps.append({"x": _prep_x(x[b]), "wvqk": wvqk, "wo": wo})
    res = run_bass_kernel_spmd(nc, in_maps, core_ids=list(range(B)))
    out = np.empty((B, F, T), dtype=np.float32)
    for b in range(B):
        out[b] = _unpack_out(res.results[b]["out"])
    return out


def run_traced(x, w_qkv, w_out):
    """Like kernel() but traced; returns (out, BassKernelResults)."""
    import concourse.bass_utils as bu
    bu.upload_artifacts = lambda tmpdir: tmpdir
    x = np.asarray(x, dtype=np.float32)
    nc = _get_nc()
    wvqk, wo = _prep_weights(np.asarray(w_qkv, np.float32), np.asarray(w_out, np.float32))
    in_maps = []
    for b in range(B):
        in_maps.append({"x": _prep_x(x[b]), "wvqk": wvqk, "wo": wo})
    res = run_bass_kernel_spmd(nc, in_maps, core_ids=list(range(B)), trace=True)
    out = np.empty((B, F, T), dtype=np.float32)
    for b in range(B):
        out[b] = _unpack_out(res.results[b]["out"])
    return out, res
